# revision 13
# baseline (speedup 1.0000x reference)
"""GQA causal attention (B=2,S=2048,D=2048,N=8,K=1,H=256) on 8 TRN2 NeuronCores.

Sharding: data-parallel over batch (cores 0-3 batch 0, cores 4-7 batch 1) x
zigzag query-block parallel within each batch. Rank r owns q-blocks (r, 7-r)
of 256 rows each, which balances causal-attention work exactly across ranks.
K/V are computed for each core's own rows, RoPE'd, and all-gathered within
the 4-core batch group. The output projection is local to each core's rows,
so no output reduction is needed.

fp8 quantization (matching the reference's E4M3 casts) is done host-side for
inputs/weights and on-device (bf16 -> e4m3) for probs and encoded.
"""
import sys
import numpy as np
import ml_dtypes

B, S, D, N, KV, H = 2, 2048, 2048, 8, 1, 256
HALF = H // 2  # 128
P = 128
BIG_NEG = -2.3819763e+38
NCORES = 8
E4FN = ml_dtypes.float8_e4m3fn
E4 = ml_dtypes.float8_e4m3
BF16 = ml_dtypes.bfloat16

_CACHE = {}


# ---------------------------------------------------------------- host helpers

def _ensure_antenv_hooks():
    """bass_utils imports antenv.axon_hooks in the trace path; stub it if the
    container's antenv lacks it so tracing degrades instead of crashing."""
    try:
        import antenv.axon_hooks  # noqa: F401
    except Exception:
        import types
        mod = types.ModuleType("antenv.axon_hooks")
        mod._hook = None
        mod.set_axon_ntff_profile_hook = lambda h: setattr(mod, "_hook", h)
        mod.get_axon_ntff_profile_hook = lambda: mod._hook
        sys.modules["antenv.axon_hooks"] = mod


def _patch_bass_utils():
    import concourse.bass_utils as bass_utils
    if getattr(bass_utils, "_attn_upload_patched", False):
        return
    orig = bass_utils.upload_artifacts

    def _safe_upload(tmpdir):
        try:
            return orig(tmpdir)
        except Exception:
            return tmpdir

    bass_utils.upload_artifacts = _safe_upload
    bass_utils._attn_upload_patched = True


def _patch_tile_drain():
    """This walrus build accepts only one sync-wait per Drain; split the Tile
    tail-drain's waits across multiple drain instructions."""
    import concourse.mybir as mybir
    import concourse.tile as tile
    from concourse.vector_clock import ScopedClock
    if getattr(tile.TileContext, "_attn_drain_patched", False):
        return

    def _drain_and_barrier(self, tick_clock, wait_clock):
        nc = self.nc
        drain_inst = nc.sync.drain()
        wait_clock.add_sem_waits(
            drain_inst.ins, ScopedClock({None: tick_clock.global_clock})
        )
        si = drain_inst.ins.sync_info
        if si is not None and si.on_wait and len(si.on_wait) > 1:
            waits = list(si.on_wait)
            drain_inst.ins.sync_info = mybir.SyncInfo(
                on_wait=waits[:1], on_update=list(si.on_update or [])
            )
            for w in waits[1:]:
                d2 = nc.sync.drain()
                d2.ins.sync_info = mybir.SyncInfo(on_wait=[w], on_update=[])
        nc.all_engine_barrier()
        assert self.sems is not None
        popped = nc._tile_sem_poison_stack.pop()
        assert popped is self._sem_poison
        nc.clear_and_free_semaphores(list(self.sems.allocated().values()))
        nc.all_engine_barrier()

    tile.TileContext._drain_and_barrier = _drain_and_barrier
    tile.TileContext._attn_drain_patched = True


def _split_multi_waits(nc, mybir):
    """Walrus here accepts one sync-wait per instruction; hoist extra waits
    onto same-engine NOPs inserted just before the instruction."""
    cnt = 0
    for f in nc.m.functions:
        for bb in f.blocks:
            insts = bb.instructions
            out = []
            changed = False
            for inst in insts:
                si = inst.sync_info
                if si is not None and si.on_wait and len(si.on_wait) > 1:
                    waits = list(si.on_wait)
                    for w in waits[:-1]:
                        nop = mybir.InstNoOp(name=f"I-wsplit-{cnt}", ins=[], outs=[])
                        cnt += 1
                        nop.engine = inst.engine
                        nop.sync_info = mybir.SyncInfo(on_wait=[w], on_update=[])
                        out.append(nop)
                    inst.sync_info = mybir.SyncInfo(
                        on_wait=[waits[-1]], on_update=list(si.on_update or [])
                    )
                    changed = True
                out.append(inst)
            if changed:
                bb.instructions = out
    return cnt


def _rank_rows(r):
    lo, hi = r, 7 - r
    return list(range(256 * lo, 256 * lo + 256)) + list(range(256 * hi, 256 * hi + 256))


def _gslot(s):
    """abs key 128-tile s -> gathered 128-slot index (AllGather is rank-major;
    rank rr contributes blocks (rr, 7-rr))."""
    bb = s // 2
    rr = bb if bb < 4 else 7 - bb
    off = (0 if bb < 4 else 2) + (s % 2)
    return 4 * rr + off


def _tile_geom(rp, s):
    """For a 256-row q-panel whose abs block index is rp, key tile s:
    (c0, cw, tri_off) — valid col range [c0, c0+cw) within the panel's 256
    cols, and the col offset of the 128x128 causal triangle (or None)."""
    if s < 2 * rp:
        return 0, 256, None
    if s == 2 * rp:
        return 0, 256, 0
    assert s == 2 * rp + 1
    return 128, 128, 128


# ---------------------------------------------------------------- graph build

def _build(bass, mybir, tile):
    from concourse.masks import make_identity

    f32 = mybir.dt.float32
    f32r = mybir.dt.float32r
    bf16 = mybir.dt.bfloat16
    f8 = mybir.dt.float8e4

    nc = bass.Bass(num_devices=NCORES)

    xt8_e = nc.dram_tensor("xt8", [D, 512], f8, kind="ExternalInput")
    wq8_e = nc.dram_tensor("wq8", [N, D, H], f8, kind="ExternalInput")
    wk8_e = nc.dram_tensor("wk8", [D, H], f8, kind="ExternalInput")
    wv8_e = nc.dram_tensor("wv8", [D, H], f8, kind="ExternalInput")
    wout8_e = nc.dram_tensor("wout8", [N * H, D], f8, kind="ExternalInput")
    cosq_e = nc.dram_tensor("cosq", [HALF, 512], f32, kind="ExternalInput")
    sinq_e = nc.dram_tensor("sinq", [HALF, 512], f32, kind="ExternalInput")
    cosk_e = nc.dram_tensor("cosk", [HALF, 512], f32, kind="ExternalInput")
    sink_e = nc.dram_tensor("sink", [HALF, 512], f32, kind="ExternalInput")
    tri_e = nc.dram_tensor("tri", [128, 128], f32, kind="ExternalInput")
    ones_e = nc.dram_tensor("ones", [P, 128], f32r, kind="ExternalInput")

    out_e = nc.dram_tensor("out", [512, D], bf16, kind="ExternalOutput")
    kout_e = nc.dram_tensor("k_out", [512, H], bf16, kind="ExternalOutput")
    vout_e = nc.dram_tensor("v_out", [512, H], bf16, kind="ExternalOutput")

    KVIN = 128 * 2 * 512 + 128 * 4 * 256  # kTr + v, bf16 elements
    nkt = 128 * 2 * 512

    with tile.TileContext(nc) as tc:
        import contextlib
        with contextlib.ExitStack() as ctx:
            sb = ctx.enter_context(tc.tile_pool(name="sb", bufs=1))
            dram = ctx.enter_context(tc.tile_pool(name="dram", bufs=1, space="DRAM"))

            # ---- persistent SBUF tensors
            xt_sb = sb.tile([P, 16, 512], f8, tag="xt")
            wk_sb = sb.tile([P, 16, H], f8, tag="wk")
            wv_sb = sb.tile([P, 16, H], f8, tag="wv")
            wout_sb = sb.tile([P, 16, D], f8, tag="wout")
            cosq_sb = sb.tile([P, 512], f32, tag="cosq")
            sinq_sb = sb.tile([P, 512], f32, tag="sinq")
            cosk_sb = sb.tile([P, 512], f32, tag="cosk")
            sink_sb = sb.tile([P, 512], f32, tag="sink")
            tri_sb = sb.tile([P, 128], f32, tag="tri")
            ident_sb = sb.tile([P, 128], bf16, tag="ident")
            ones_sb = sb.tile([P, 128], f32r, tag="ones")
            kT_all = sb.tile([P, 2, 2048], bf16, tag="kT_all")
            v_all = sb.tile([P, 16, H], bf16, tag="v_all")
            v8_all = sb.tile([P, 16, H], f8, tag="v8_all")
            qTr_all = sb.tile([P, N, 2, 512], bf16, tag="qTr")
            enc8_all = sb.tile([P, 16, 512], f8, tag="enc8")
            exp_sb = sb.tile([P, 2, 16, 256], f32r, tag="exp")
            pbf_sb = sb.tile([P, 2, 16, 256], bf16, tag="pbf")
            p8_sb = sb.tile([P, 2, 16, 256], f8, tag="p8")
            rbc_sb = sb.tile([P, 512], f32, tag="rbc")
            srow = sb.tile([1, 512], f32r, tag="srow")
            encT = sb.tile([P, 2, 512], bf16, tag="encT")

            nc.sync.dma_start(xt_sb[:], xt8_e.rearrange("(o p) t -> p o t", p=P))
            nc.sync.dma_start(wk_sb[:], wk8_e.rearrange("(o p) h -> p o h", p=P))
            nc.sync.dma_start(wv_sb[:], wv8_e.rearrange("(o p) h -> p o h", p=P))
            nc.sync.dma_start(wout_sb[:], wout8_e.rearrange("(o p) d -> p o d", p=P))
            nc.sync.dma_start(cosq_sb[:], cosq_e[:])
            nc.sync.dma_start(sinq_sb[:], sinq_e[:])
            nc.sync.dma_start(cosk_sb[:], cosk_e[:])
            nc.sync.dma_start(sink_sb[:], sink_e[:])
            nc.sync.dma_start(tri_sb[:], tri_e[:])
            nc.sync.dma_start(ones_sb[:], ones_e[:])
            make_identity(nc, ident_sb[:])

            # ================= phase 0: KV proj + rope + gather =================
            kraw = sb.tile([P, 2, 512], bf16, tag="kraw")
            kTr = sb.tile([P, 2, 512], bf16, tag="kTr")
            v_sb = sb.tile([P, 4, H], bf16, tag="vsb")
            k_sb = sb.tile([P, 4, H], bf16, tag="ksb")
            t1 = sb.tile([P, 512], f32, tag="t1")
            t2 = sb.tile([P, 512], f32, tag="t2")

            with tc.tile_pool(name="ps_kv", bufs=2, space="PSUM") as ps_kv:
                for j in range(2):
                    psk = ps_kv.tile([P, 512], f32, tag="psk")
                    for t in range(16):
                        nc.tensor.matmul(
                            psk[:], wk_sb[:, t, 128 * j:128 * j + 128],
                            xt_sb[:, t, :], start=(t == 0), stop=(t == 15))
                    nc.vector.tensor_copy(kraw[:, j, :], psk[:])
                # rope on k
                nc.vector.tensor_mul(t1[:], kraw[:, 0, :], cosk_sb[:])
                nc.vector.tensor_mul(t2[:], kraw[:, 1, :], sink_sb[:])
                nc.vector.tensor_sub(kTr[:, 0, :], t1[:], t2[:])
                nc.vector.tensor_mul(t1[:], kraw[:, 1, :], cosk_sb[:])
                nc.vector.tensor_mul(t2[:], kraw[:, 0, :], sink_sb[:])
                nc.vector.tensor_add(kTr[:, 1, :], t1[:], t2[:])
                # v proj
                for tt in range(4):
                    psv = ps_kv.tile([P, H], f32, tag="psv")
                    for t in range(16):
                        nc.tensor.matmul(
                            psv[:], xt_sb[:, t, 128 * tt:128 * tt + 128],
                            wv_sb[:, t, :], start=(t == 0), stop=(t == 15))
                    nc.vector.tensor_copy(v_sb[:, tt, :], psv[:])
                # k_out = kTr^T via PE transpose
                for tt in range(4):
                    for j in range(2):
                        pst = ps_kv.tile([P, 128], bf16, tag="pst")
                        nc.tensor.transpose(
                            pst[:], kTr[:, j, 128 * tt:128 * tt + 128], ident_sb[:])
                        nc.vector.tensor_copy(
                            k_sb[:, tt, 128 * j:128 * j + 128], pst[:])

            nc.sync.dma_start(vout_e.rearrange("(a p) h -> p a h", p=P), v_sb[:])
            nc.sync.dma_start(kout_e.rearrange("(a p) h -> p a h", p=P), k_sb[:])

            kv_in = dram.tile([KVIN], bf16)
            kv_out = dram.tile([4 * KVIN], bf16)
            nc.sync.dma_start(
                kv_in[0:nkt].rearrange("(p a t) -> p a t", p=P, a=2), kTr[:])
            nc.sync.dma_start(
                kv_in[nkt:KVIN].rearrange("(p a h) -> p a h", p=P, a=4), v_sb[:])
            nc.gpsimd.collective_compute(
                "AllGather", mybir.AluOpType.bypass,
                replica_groups=[[0, 1, 2, 3], [4, 5, 6, 7]],
                ins=[kv_in.opt()], outs=[kv_out.opt()])
            for rr in range(4):
                base = rr * KVIN
                nc.sync.dma_start(
                    kT_all[:, :, 512 * rr:512 * rr + 512],
                    kv_out[base:base + nkt].rearrange("(p a t) -> p a t", p=P, a=2))
                nc.sync.dma_start(
                    v_all[:, 4 * rr:4 * rr + 4, :],
                    kv_out[base + nkt:base + KVIN].rearrange(
                        "(p a h) -> p a h", p=P, a=4))
            # reference quantizes V to e4m3 for the AV einsum
            nc.scalar.activation(
                v8_all[:], v_all[:], mybir.ActivationFunctionType.Copy)

            # ================= phases 1+2: Q proj + attention ===================
            with (
                tc.tile_pool(name="wq", bufs=2) as wqp,
                tc.tile_pool(name="ps_attn", bufs=1, space="PSUM") as psp,
                tc.tile_pool(name="ps_q", bufs=1, space="PSUM") as ps_q,
            ):
                sums_ps = psp.tile([1, 512], f32, tag="sums")
                lgs = (psp.tile([P, 2, 256], f32, tag="lg0", name="lg0"),
                       psp.tile([P, 2, 256], f32, tag="lg1", name="lg1"))
                av0 = psp.tile([P, 512], f32, tag="av0")
                av1 = psp.tile([P, 512], f32, tag="av1")
                lnbc_ps = psp.tile([P, 512], f32, tag="lnbc")

                for h in range(N):
                    wq_sb = wqp.tile([P, 16, H], f8, tag="wq")
                    nc.sync.dma_start(
                        wq_sb[:], wq8_e[h].rearrange("(o p) hh -> p o hh", p=P))
                    qraw = sb.tile([P, 2, 512], bf16, tag="qraw")
                    for j in range(2):
                        psq = ps_q.tile([P, 512], f32, tag="psq")
                        for t in range(16):
                            nc.tensor.matmul(
                                psq[:], wq_sb[:, t, 128 * j:128 * j + 128],
                                xt_sb[:, t, :], start=(t == 0), stop=(t == 15))
                        nc.vector.tensor_copy(qraw[:, j, :], psq[:])
                    tq1 = sb.tile([P, 512], f32, tag="tq1")
                    tq2 = sb.tile([P, 512], f32, tag="tq2")
                    nc.vector.tensor_mul(tq1[:], qraw[:, 0, :], cosq_sb[:])
                    nc.vector.tensor_mul(tq2[:], qraw[:, 1, :], sinq_sb[:])
                    nc.vector.tensor_sub(qTr_all[:, h, 0, :], tq1[:], tq2[:])
                    nc.vector.tensor_mul(tq1[:], qraw[:, 1, :], cosq_sb[:])
                    nc.vector.tensor_mul(tq2[:], qraw[:, 0, :], sinq_sb[:])
                    nc.vector.tensor_add(qTr_all[:, h, 1, :], tq1[:], tq2[:])

                # condition must be valid on every engine so each engine
                # emits its own conditional branch
                pid_regs = []
                for ename, eng in nc.engines.items():
                    tmp = eng.alloc_register(f"pid_{ename}")
                    eng.reg_load(tmp, nc.partition_id_tensor[0:1, 0:1])
                    pid_regs.append(tmp)
                pid = nc.snap(
                    bass.RegisterHandles(pid_regs), min_val=0, max_val=NCORES - 1)
                for r in range(4):
                    with tc.If(pid % 4 == r):
                        _emit_attention(
                            nc, tc, r, qTr_all, kT_all, v8_all, tri_sb, ones_sb,
                            exp_sb, pbf_sb, p8_sb, rbc_sb, srow, lnbc_ps, encT,
                            enc8_all, sums_ps, lgs, av0, av1,
                            mybir, f32, f32r)

            # ================= phase 3: out projection ==========================
            with (
                tc.tile_pool(name="ps_o", bufs=2, space="PSUM") as ps_o,
                tc.tile_pool(name="outsb", bufs=3) as outp,
            ):
                for dg in range(4):
                    for tt in range(4):
                        pso = ps_o.tile([P, 512], f32, tag="pso")
                        for kk in range(16):
                            nc.tensor.matmul(
                                pso[:], enc8_all[:, kk, 128 * tt:128 * tt + 128],
                                wout_sb[:, kk, 512 * dg:512 * dg + 512],
                                start=(kk == 0), stop=(kk == 15))
                        out_bf = outp.tile([P, 512], bf16, tag="outbf")
                        nc.vector.tensor_copy(out_bf[:], pso[:])
                        nc.sync.dma_start(
                            out_e[128 * tt:128 * tt + 128, 512 * dg:512 * dg + 512],
                            out_bf[:])

    _split_multi_waits(nc, mybir)
    return nc


def _emit_attention(nc, tc, r, qTr_all, kT_all, v_all, tri_sb, ones_sb,
                    exp_sb, pbf_sb, p8_sb, rbc_sb, srow, lnbc_ps, encT,
                    enc8_all, sums_ps, lgs, av0, av1, mybir, f32, f32r):
    Exp = mybir.ActivationFunctionType.Exp
    Log = mybir.ActivationFunctionType.Ln
    Copy = mybir.ActivationFunctionType.Copy
    GROUP = 2
    panels = [(0, r), (1, 7 - r)]

    for h in range(N):
        gctr = 0
        first_mm = True
        for panel, rp in panels:
            T = 2 * rp + 2
            base = 256 * panel
            for g in range(0, T, GROUP):
                gsz = min(GROUP, T - g)
                lg = lgs[gctr % 2]
                gctr += 1
                for si in range(gsz):
                    s = g + si
                    c0, cw, tri_off = _tile_geom(rp, s)
                    gs = _gslot(s)
                    for j in range(2):
                        nc.tensor.matmul(
                            lg[:, si, c0:c0 + cw],
                            kT_all[:, j, 128 * gs:128 * gs + 128],
                            qTr_all[:, h, j, base + c0:base + c0 + cw],
                            start=(j == 0), stop=(j == 1))
                    if tri_off is not None:
                        nc.vector.tensor_add(
                            lg[:, si, tri_off:tri_off + 128],
                            lg[:, si, tri_off:tri_off + 128], tri_sb[:])
                nc.scalar.activation(
                    exp_sb[:, panel, g:g + gsz, :], lg[:, 0:gsz, :], Exp)
                for si in range(gsz):
                    s = g + si
                    c0, cw, _ = _tile_geom(rp, s)
                    nc.tensor.matmul(
                        sums_ps[0:1, base + c0:base + c0 + cw],
                        ones_sb[:, 0:1],
                        exp_sb[:, panel, s, c0:c0 + cw],
                        start=first_mm,
                        stop=(panel == 1 and s == T - 1))
                    first_mm = False
        # broadcast S with a K=1 matmul, then exact elementwise 1/S on DVE
        nc.scalar.activation(srow[:], sums_ps[0:1, :], Copy)
        nc.tensor.matmul(lnbc_ps[:], ones_sb[0:1, :], srow[:],
                         start=True, stop=True)
        nc.vector.reciprocal(rbc_sb[:], lnbc_ps[:])
        # normalize -> bf16 -> e4m3
        for panel, rp in panels:
            T = 2 * rp + 2
            base = 256 * panel
            for g in range(0, T, GROUP):
                gsz = min(GROUP, T - g)
                nc.vector.tensor_mul(
                    pbf_sb[:, panel, g:g + gsz, :],
                    exp_sb[:, panel, g:g + gsz, :].bitcast(f32),
                    rbc_sb[:, None, base:base + 256].to_broadcast((P, gsz, 256)))
            nc.scalar.activation(
                p8_sb[:, panel, 0:T, :], pbf_sb[:, panel, 0:T, :], Copy)
        # AV: encT[h-half, q] accumulated over key tiles
        avs = (av0, av1)
        first_av = True
        for panel, rp in panels:
            T = 2 * rp + 2
            base = 256 * panel
            for s in range(T):
                c0, cw, _ = _tile_geom(rp, s)
                gs = _gslot(s)
                last = (panel == 1 and s == T - 1)
                for j2 in range(2):
                    nc.tensor.matmul(
                        avs[j2][:, base + c0:base + c0 + cw],
                        v_all[:, gs, 128 * j2:128 * j2 + 128],
                        p8_sb[:, panel, s, c0:c0 + cw],
                        start=first_av, stop=last)
                first_av = False
        for j2 in range(2):
            nc.vector.tensor_copy(encT[:, j2, :], avs[j2][:])
        nc.scalar.activation(enc8_all[:, 2 * h:2 * h + 2, :], encT[:], Copy)


# ---------------------------------------------------------------- numpy fallback

def _numpy_fallback(x, positions, attn_mask, w_q, w_kv, w_out):
    half = H // 2
    freq = (2.0 / H) * np.arange(half, dtype=np.float32)
    ts = (10000.0 ** freq).astype(np.float32)
    rad = positions.astype(np.float32)[..., None] / ts[None, None, :]
    sin_t, cos_t = np.sin(rad).astype(np.float32), np.cos(rad).astype(np.float32)

    def e4(a):
        return a.astype(E4FN)

    def mmf(a8, b8):
        return np.matmul(a8.astype(np.float32), b8.astype(np.float32))

    x8 = e4(x)
    wq8, wkv8, wout8 = e4(w_q), e4(w_kv), e4(w_out)
    Bi, Si = x.shape[0], x.shape[1]
    outs = np.zeros((Bi, Si, D), BF16)
    ks = np.zeros((Bi, Si, 1, H), BF16)
    vs = np.zeros((Bi, Si, 1, H), BF16)
    for b in range(Bi):
        q = np.stack([mmf(x8[b], wq8[n]).astype(BF16) for n in range(N)], 1)
        k_ = mmf(x8[b], wkv8[0, 0]).astype(BF16)
        v_ = mmf(x8[b], wkv8[1, 0]).astype(BF16)

        def rope(t, c, s_):
            t1 = t[..., :half].astype(np.float32)
            t2 = t[..., half:].astype(np.float32)
            return np.concatenate(
                [t1 * c - t2 * s_, t2 * c + t1 * s_], -1).astype(BF16)

        qr = rope(q, cos_t[b][:, None, :], sin_t[b][:, None, :])
        qr = (qr.astype(np.float32) * (H ** -0.5)).astype(BF16)
        kr = rope(k_, cos_t[b], sin_t[b])
        ks[b, :, 0, :], vs[b, :, 0, :] = kr, v_
        enc = np.zeros((Si, N, H), BF16)
        m = attn_mask[b, 0]
        for n in range(N):
            lo = np.matmul(qr[:, n].astype(np.float32), kr.astype(np.float32).T)
            lo = np.where(m, lo, BIG_NEG)
            e = np.exp(lo)
            probs = (e / e.sum(-1, keepdims=True)).astype(BF16)
            enc[:, n, :] = mmf(e4(probs), e4(v_)).astype(BF16)
        outs[b] = mmf(e4(enc.reshape(Si, N * H)),
                      wout8.reshape(N * H, D)).astype(BF16)
    return outs, ks, vs


# ---------------------------------------------------------------- entry point

LAST_EXEC_TIME_NS = None


def kernel(x, positions, attn_mask, w_q, w_kv, w_out):
    global LAST_EXEC_TIME_NS
    x = np.asarray(x)
    positions = np.asarray(positions)
    attn_mask = np.asarray(attn_mask)
    w_q = np.asarray(w_q, dtype=np.float32)
    w_kv = np.asarray(w_kv, dtype=np.float32)
    w_out = np.asarray(w_out, dtype=np.float32)
    if x.dtype != BF16:
        x = x.astype(BF16)

    causal = np.tril(np.ones((S, S), bool))
    if (x.shape != (B, S, D) or not all(
            np.array_equal(attn_mask[b, 0], causal) for b in range(B))):
        return _numpy_fallback(x, positions, attn_mask, w_q, w_kv, w_out)

    _ensure_antenv_hooks()
    _patch_bass_utils()
    _patch_tile_drain()
    import concourse.bass as bass
    import concourse.mybir as mybir
    import concourse.tile as tile
    from concourse.bass_utils import run_bass_kernel_spmd

    if "nc" not in _CACHE:
        _CACHE["nc"] = _build(bass, mybir, tile)
    nc = _CACHE["nc"]

    # ---- host-side prep
    half = H // 2
    freq = (2.0 / H) * np.arange(half, dtype=np.float32)
    ts = (10000.0 ** freq).astype(np.float32)
    rad = positions.astype(np.float32)[..., None] / ts[None, None, :]  # [B,S,half]
    sin_t = np.sin(rad).astype(np.float32)
    cos_t = np.cos(rad).astype(np.float32)

    x8 = x.astype(E4FN).view(E4)                                  # [B,S,D]
    wq8 = np.ascontiguousarray(w_q.astype(E4FN).view(E4))         # [N,D,H]
    wk8 = np.ascontiguousarray(w_kv[0, 0].astype(E4FN).view(E4))  # [D,H]
    wv8 = np.ascontiguousarray(w_kv[1, 0].astype(E4FN).view(E4))  # [D,H]
    wout8 = np.ascontiguousarray(
        w_out.reshape(N * H, D).astype(E4FN).view(E4))            # [NH,D]

    tri = np.zeros((128, 128), np.float32)
    tri[np.tril_indices(128, -1)] = BIG_NEG  # [k,q]: masked iff k > q

    scale = np.float32(1.0 / np.sqrt(H))
    in_maps = []
    for c in range(NCORES):
        b, r = c // 4, c % 4
        rows = _rank_rows(r)
        in_maps.append({
            "xt8": np.ascontiguousarray(x8[b][rows].T),
            "wq8": wq8, "wk8": wk8, "wv8": wv8, "wout8": wout8,
            "cosq": np.ascontiguousarray(cos_t[b][rows].T * scale),
            "sinq": np.ascontiguousarray(sin_t[b][rows].T * scale),
            "cosk": np.ascontiguousarray(cos_t[b][rows].T),
            "sink": np.ascontiguousarray(sin_t[b][rows].T),
            "tri": tri, "ones": np.ones((P, 128), np.float32),
        })

    res = run_bass_kernel_spmd(nc, in_maps, list(range(NCORES)))
    LAST_EXEC_TIME_NS = res.exec_time_ns

    out = np.zeros((B, S, D), BF16)
    k_full = np.zeros((B, S, 1, H), BF16)
    v_full = np.zeros((B, S, 1, H), BF16)
    for c in range(NCORES):
        b, r = c // 4, c % 4
        rows = _rank_rows(r)
        out[b, rows] = res.results[c]["out"]
        k_full[b, rows, 0] = res.results[c]["k_out"]
        v_full[b, rows, 0] = res.results[c]["v_out"]
    return out, k_full, v_full
